# revision 36
# baseline (speedup 1.0000x reference)
"""CAPGNN message-passing kernel for 8 Trainium2 NeuronCores.

Sharding (per hint): nodes partitioned across the 8 cores; per core the
destinations are degree-sorted into 98 tiles of 128 (one dest per SBUF
partition) with non-self edges in an ELL layout along the free axis, split
into 4 source ranges of 25088 rows so dma_gather's int16 indices can address
the replicated tables. Self loops are excluded from the edge stream and
folded into the per-step update as a per-dest scalar (selfa). The gathered
h table holds dis (.) h (deg^-1/2 pre-scaled), so the edge affinity drops
its dis[col] factor; (1-ALPHA) is folded into the affinity as well. The
attention pass gathers 512B rows [K | dis (.) V] and its second half feeds
propagation step 0 directly (one fewer gather pass). Gather index tables are
SBUF-resident across all steps. Each remaining step gathers h[col] with
batched dma_gather, multiplies by the resident affinity, segment-reduces on
DVE, updates h, and AllGathers the re-scaled shard. Dense weights are
replicated.
"""
import numpy as np

from concourse import bacc, mybir, tile, library_config
from concourse import bass_utils

F32 = mybir.dt.float32
I16 = mybir.dt.int16

N = 100000
F = 128
D = 64
AU = 64
T = 10
ALPHA = 0.1
BETA = 0.3
NCORES = 8
NSHARD = 12500
SLOTS = 12544          # 98 * 128 (44 phantom slots)
NT = 98
P = 128
NRANGE = 4
RSZ = 2 * SLOTS        # 25088 rows per gather range (int16-addressable)
PAD_LOCAL = SLOTS + 12543  # zero pad row: odd core of the pair, slot 12543
CH = 64                # max ELL columns per propagation chunk
CH_ATT = 32            # max cols per attention sub-call (KG rows are 512B)
NEG = -1.0e30
TRACE = False
LAST_RESULTS = None
SIM_MODE = False
SKIP = set()
T_LOOP = T


def _preprocess(edge_index):
    row = edge_index[0].astype(np.int64)
    col = edge_index[1].astype(np.int64)
    nonself = row != col
    row = row[nonself]
    col = col[nonself]
    # deg matches the reference (self loop added there)
    deg = np.bincount(row, minlength=N).astype(np.int64) + 1

    # greedy node->pair assignment balancing every dest's per-range counts
    order_c = np.argsort(col, kind="stable")
    dests_by_src = row[order_c]
    src_off = np.zeros(N + 1, np.int64)
    np.cumsum(np.bincount(col, minlength=N), out=src_off[1:])
    quarter = ((deg - 1 + NRANGE - 1) // NRANGE).astype(np.int32)
    # exponential excess penalty: flattening per-dest per-range counts
    WT = np.zeros(24, np.int64)
    for i in range(24):
        e = i - 8
        WT[i] = 0 if e < 0 else 4 ** min(e, 8)
    cnt = np.zeros((N, NRANGE), np.int32)
    pair_of = np.full(N, -1, np.int32)
    fill = np.zeros(NRANGE, np.int64)
    cap = 2 * NSHARD
    out_deg = np.diff(src_off)
    visit = np.argsort(-out_deg, kind="stable")
    for v in visit:
        ds = dests_by_src[src_off[v]:src_off[v + 1]]
        if ds.size:
            exc = np.clip(cnt[ds] + 1 - quarter[ds, None] + 8, 0, 23)
            over = WT[exc].sum(0)
        else:
            over = np.zeros(NRANGE, np.int64)
        score = over * (cap + 1) + fill
        score = score + np.where(fill >= cap, np.int64(1) << 50, 0)
        r = int(np.argmin(score))
        pair_of[v] = r
        fill[r] += 1
        if ds.size:
            cnt[ds, r] += 1

    core_of = np.full(N, -1, np.int32)
    for r in range(NRANGE):
        vs = np.where(pair_of == r)[0]
        vs = vs[np.argsort(-deg[vs], kind="stable")]
        core_of[vs[0::2]] = 2 * r
        core_of[vs[1::2]] = 2 * r + 1

    # order dests within each core so tiles group similar count vectors
    # (minimizes per-tile per-range maxima -> less ELL padding)
    cnts = np.bincount(row * NRANGE + pair_of[col],
                       minlength=N * NRANGE).reshape(N, NRANGE)
    mx = cnts.max(1)
    am = cnts.argmax(1)
    ghat = np.zeros(N, np.int64)
    tau_of = np.zeros(N, np.int64)
    p_of = np.zeros(N, np.int64)
    nodes_by_rank = np.full((NCORES, SLOTS), -1, np.int64)
    for c in range(NCORES):
        vs = np.where(core_of == c)[0]
        vs = vs[np.lexsort((-cnts[vs, 3], -cnts[vs, 2], -cnts[vs, 1],
                            -cnts[vs, 0], am[vs], -mx[vs]))]
        k = np.arange(vs.size)
        tau_of[vs] = k // P
        p_of[vs] = k % P
        ghat[vs] = c * SLOTS + p_of[vs] * NT + tau_of[vs]
        nodes_by_rank[c, :vs.size] = vs

    gcol = ghat[col]
    e_core = core_of[row]
    e_tau = tau_of[row]
    e_p = p_of[row]
    e_r = gcol // RSZ
    e_lcol = (gcol % RSZ).astype(np.int64)

    key = ((e_core * NT + e_tau) * P + e_p) * NRANGE + e_r
    nkeys = NCORES * NT * P * NRANGE
    counts = np.bincount(key, minlength=nkeys)
    starts = np.zeros(nkeys + 1, np.int64)
    np.cumsum(counts, out=starts[1:])
    eo = np.argsort(key, kind="stable")
    j_sorted = np.arange(key.size, dtype=np.int64) - starts[key[eo]]
    j_of = np.zeros(key.size, np.int64)
    j_of[eo] = j_sorted

    cnt4 = counts.reshape(NCORES, NT, P, NRANGE)
    M = np.maximum(cnt4.max(axis=(0, 2)), 1).astype(np.int64)  # [NT, NRANGE]

    tcolbase = np.zeros((NT, NRANGE), np.int64)
    acc = 0
    for t in range(NT):
        for r in range(NRANGE):
            tcolbase[t, r] = acc
            acc += int(M[t, r])
    Call = int(acc)

    chunks = []
    cur, cur_w = [], 0
    for t in range(NT):
        w = int(M[t].sum())
        if cur and cur_w + w > CH:
            chunks.append(cur)
            cur, cur_w = [], 0
        cur.append(t)
        cur_w += w
    if cur:
        chunks.append(cur)

    gcolbase = np.zeros((NT, NRANGE), np.int64)
    acc = 0
    chunk_meta = []  # (col0, cols, tcol0, [(r, [(tau, loff_in_r, M)], rcol0, rcols)])
    for ch in chunks:
        col0 = acc
        rinfo = []
        for r in range(NRANGE):
            rcol0 = acc
            blocks = []
            for t in ch:
                gcolbase[t, r] = acc
                blocks.append((t, acc - rcol0, int(M[t, r])))
                acc += int(M[t, r])
            rinfo.append((r, blocks, rcol0, acc - rcol0))
        chunk_meta.append((col0, acc - col0, int(tcolbase[ch[0], 0]), rinfo))
    assert acc == Call

    col16 = np.full((NCORES, P, Call), PAD_LOCAL, np.int16)
    maskNeg = np.full((NCORES, P, Call), NEG, np.float32)
    e_gc = gcolbase[e_tau, e_r] + j_of
    e_tc = tcolbase[e_tau, e_r] + j_of
    col16[e_core, e_p, e_gc] = e_lcol.astype(np.int16)
    maskNeg[e_core, e_p, e_tc] = 0.0
    # phantom rows (tile 97, partitions 84.. on every core): unmask their
    # first r0 pad column so softmax Z = 1 instead of 0 (no NaN).
    maskNeg[:, 84:, tcolbase[NT - 1, 0]] = 0.0

    idxw = np.zeros((NCORES, P, 8 * Call), np.int16)
    for c in range(NCORES):
        for (col0, cols, tcol0, rinfo) in chunk_meta:
            for (r, blocks, rcol0, rcols) in rinfo:
                A = col16[c][:, rcol0:rcol0 + rcols]
                flat = A.T.reshape(-1)
                w16 = flat.reshape(-1, 16).T
                idxw[c][:, 8 * rcol0:8 * (rcol0 + rcols)] = np.tile(w16, (8, 1))

    deg_ell = np.ones((NCORES, P, NT), np.float32)
    for c in range(NCORES):
        vs = nodes_by_rank[c]
        real = vs >= 0
        k = np.arange(SLOTS)
        deg_ell[c, k[real] % P, k[real] // P] = deg[vs[real]].astype(np.float32)

    return dict(deg=deg, core_of=core_of, nodes_by_rank=nodes_by_rank,
                M=M, Call=Call, chunks=chunks, chunk_meta=chunk_meta,
                tcolbase=tcolbase, gcolbase=gcolbase,
                col16=col16, maskNeg=maskNeg, idxw=idxw, deg_ell=deg_ell)


def _build_kernel(pp):
    Call = pp["Call"]
    chunk_meta = pp["chunk_meta"]
    tcolbase = pp["tcolbase"]
    gcolbase = pp["gcolbase"]
    M = pp["M"]
    TMAX = max(len(ch) for ch in pp["chunks"])

    nc = bacc.Bacc("TRN2", target_bir_lowering=False, debug=False,
                   num_devices=1 if SIM_MODE else NCORES)

    xT_in = nc.dram_tensor("xT", [P, SLOTS], F32, kind="ExternalInput")
    idx_in = nc.dram_tensor("idxw", [P, 8 * Call], I16, kind="ExternalInput")
    mask_in = nc.dram_tensor("maskNeg", [P, Call], F32, kind="ExternalInput")
    deg_in = nc.dram_tensor("deg_ell", [P, NT], F32, kind="ExternalInput")
    W1_in = nc.dram_tensor("W1", [F, 128], F32, kind="ExternalInput")
    W2_in = nc.dram_tensor("W2", [128, D], F32, kind="ExternalInput")
    Wq_in = nc.dram_tensor("Wq", [F, AU], F32, kind="ExternalInput")
    Wk_in = nc.dram_tensor("Wk", [F, AU], F32, kind="ExternalInput")
    b1_in = nc.dram_tensor("b1", [128], F32, kind="ExternalInput")
    bq_in = nc.dram_tensor("bq", [AU], F32, kind="ExternalInput")
    bk_in = nc.dram_tensor("bk", [AU], F32, kind="ExternalInput")
    b2_in = nc.dram_tensor("b2", [D], F32, kind="ExternalInput")
    att_in = nc.dram_tensor("att", [T], F32, kind="ExternalInput")
    out_dram = nc.dram_tensor("out", [P, NT * D], F32, kind="ExternalOutput")

    ag_h_in = nc.dram_tensor("ag_h_in", [SLOTS, D], F32)
    ag_K_in = nc.dram_tensor("ag_K_in", [SLOTS, 128], F32)
    # NOTE: dma_gather from Shared scratchpad faults the device; keep the
    # gather tables in Local scratchpad (AllGather to Local is supported).
    h_full = nc.dram_tensor("h_full", [NCORES * SLOTS, D], F32)
    K_full = nc.dram_tensor("K_full", [NCORES * SLOTS, 128], F32)
    RG = [list(range(NCORES))]

    AX = mybir.AxisListType.X
    OP = mybir.AluOpType
    AF = mybir.ActivationFunctionType

    with tile.TileContext(nc) as tc:
        nc.gpsimd.load_library(library_config.mlp)
        with tc.tile_pool(name="pers", bufs=1) as pers, \
             tc.tile_pool(name="work", bufs=2) as work, \
             tc.tile_pool(name="cbuf", bufs=2) as cbuf, \
             tc.tile_pool(name="psum", bufs=2, space="PSUM") as psum:

            # ---------- phase 0: weights / biases / broadcasts ----------
            W1_sb = pers.tile([F, 128], F32, tag="W1")
            W2_sb = pers.tile([128, D], F32, tag="W2")
            WqWk = pers.tile([F, 128], F32, tag="WqWk")
            b1_col = pers.tile([P, 1], F32, tag="b1")
            nc.sync.dma_start(out=W1_sb[:], in_=W1_in[:, :])
            nc.sync.dma_start(out=W2_sb[:], in_=W2_in[:, :])
            nc.sync.dma_start(out=WqWk[:, 0:AU], in_=Wq_in[:, :])
            nc.sync.dma_start(out=WqWk[:, AU:128], in_=Wk_in[:, :])
            nc.vector.tensor_scalar_mul(WqWk[:, 0:AU], WqWk[:, 0:AU], 0.125)
            nc.sync.dma_start(out=b1_col[:], in_=b1_in[:].unsqueeze(1))

            brow = pers.tile([1, 330], F32, tag="brow")
            nc.sync.dma_start(out=brow[0:1, 0:64], in_=bq_in[:].unsqueeze(0))
            nc.sync.dma_start(out=brow[0:1, 64:128], in_=bk_in[:].unsqueeze(0))
            nc.sync.dma_start(out=brow[0:1, 128:192], in_=b2_in[:].unsqueeze(0))
            nc.sync.dma_start(out=brow[0:1, 192:256], in_=b2_in[:].unsqueeze(0))
            nc.sync.dma_start(out=brow[0:1, 256:266], in_=att_in[:].unsqueeze(0))
            nc.vector.tensor_scalar_mul(brow[0:1, 0:64], brow[0:1, 0:64], 0.125)
            nc.vector.tensor_scalar_mul(brow[0:1, 192:256],
                                        brow[0:1, 192:256], ALPHA)
            bcast = pers.tile([P, 330], F32, tag="bcast")
            nc.gpsimd.partition_broadcast(bcast[:], brow[:])
            qkb = bcast[:, 0:128]
            b2b = bcast[:, 128:192]
            b2bA = bcast[:, 192:256]
            attb = bcast[:, 256:266]

            hopw = pers.tile([P, T], F32, tag="hopw")
            sc1 = pers.tile([P, 1], F32, tag="sc1")
            nc.scalar.activation(out=hopw[:], in_=attb, func=AF.Lrelu,
                                 alpha=0.2)
            nc.vector.tensor_reduce(out=sc1[:], in_=hopw[:], axis=AX,
                                    op=OP.max)
            nc.vector.tensor_scalar_mul(sc1[:], sc1[:], -1.0)
            nc.scalar.activation(out=hopw[:], in_=hopw[:], func=AF.Exp,
                                 bias=sc1[:])
            nc.vector.tensor_reduce(out=sc1[:], in_=hopw[:], axis=AX,
                                    op=OP.add)
            nc.vector.tensor_scalar_mul(sc1[:], sc1[:], 1.0 + 1e-8)
            nc.vector.reciprocal(sc1[:], sc1[:])
            nc.vector.tensor_scalar_mul(hopw[:], hopw[:], sc1[:])

            deg_sb = pers.tile([P, NT], F32, tag="deg")
            ds_sb = pers.tile([P, NT], F32, tag="ds")
            dis_sb = pers.tile([P, NT], F32, tag="dis")
            c1_sb = pers.tile([P, NT], F32, tag="c1")
            c2_sb = pers.tile([P, NT], F32, tag="c2")
            nc.sync.dma_start(out=deg_sb[:], in_=deg_in[:, :])
            nc.scalar.sqrt(out=ds_sb[:], in_=deg_sb[:])
            nc.vector.reciprocal(dis_sb[:], ds_sb[:])
            # fold (1 - ALPHA) into the edge affinity so the step update is
            # h = agg + (selfa (.) h_old + Vp)
            nc.vector.tensor_scalar_mul(c1_sb[:], dis_sb[:],
                                        BETA * (1.0 - ALPHA))
            nc.vector.tensor_scalar_mul(c2_sb[:], ds_sb[:],
                                        (1.0 - BETA) * (1.0 - ALPHA))

            # resident gather-index table (used by phase 2 and phase 3)
            idx_res = pers.tile([P, 8 * Call], I16, tag="idxres")
            nc.sync.dma_start(out=idx_res[:], in_=idx_in[:, :])

            # ---------- phase 1: dense V / Q / K ----------
            Vp = pers.tile([P, NT * D], F32, tag="Vp")
            h_stage = pers.tile([P, NT * D], F32, tag="hstage")
            Qn = pers.tile([P, NT * AU], F32, tag="big1")  # shared w/ out_acc
            qself = pers.tile([P, NT], F32, tag="qself")
            gatself = pers.tile([P, NT], F32, tag="gatself")
            selfa = pers.tile([P, NT], F32, tag="selfa")
            agKv = ag_K_in[:, :].rearrange("(p t) d -> p t d", p=P)

            for g0 in range(0, NT, 2):
                gt = min(2, NT - g0)
                xc = work.tile([P, 256], F32, tag="xc")
                nc.sync.dma_start(out=xc[:, 0:gt * P],
                                  in_=xT_in[:, g0 * P:(g0 + gt) * P])
                a_ps = psum.tile([P, 256], F32, tag="aps")
                nc.tensor.matmul(out=a_ps[:, 0:gt * P], lhsT=W1_sb[:],
                                 rhs=xc[:, 0:gt * P], start=True, stop=True)
                a1 = work.tile([P, 256], F32, tag="a1")
                nc.scalar.activation(out=a1[:, 0:gt * P], in_=a_ps[:, 0:gt * P],
                                     func=AF.Relu, bias=b1_col[:])
                for s in range(gt):
                    t = g0 + s
                    v_ps = psum.tile([P, D], F32, tag="vps")
                    nc.tensor.matmul(out=v_ps[:],
                                     lhsT=a1[:, s * P:(s + 1) * P],
                                     rhs=W2_sb[:], start=True, stop=True)
                    nc.vector.scalar_tensor_tensor(
                        out=h_stage[:, t * D:(t + 1) * D], in0=v_ps[:],
                        scalar=1.0, in1=b2b, op0=OP.mult, op1=OP.add)
                    nc.vector.scalar_tensor_tensor(
                        out=Vp[:, t * D:(t + 1) * D], in0=v_ps[:],
                        scalar=ALPHA, in1=b2bA, op0=OP.mult, op1=OP.add)
                    qk_ps = psum.tile([P, 128], F32, tag="qkps")
                    nc.tensor.matmul(out=qk_ps[:],
                                     lhsT=xc[:, s * P:(s + 1) * P],
                                     rhs=WqWk[:], start=True, stop=True)
                    qk = work.tile([P, 128], F32, tag="qk")
                    nc.vector.tensor_tensor(out=qk[:], in0=qk_ps[:], in1=qkb,
                                            op=OP.add)
                    nc.scalar.activation(out=qk[:], in_=qk[:], func=AF.Relu)
                    nc.vector.tensor_copy(out=Qn[:, t * AU:(t + 1) * AU],
                                          in_=qk[:, 0:AU])
                    qs = work.tile([P, AU], F32, tag="qs")
                    nc.vector.tensor_tensor(out=qs[:], in0=qk[:, 0:AU],
                                            in1=qk[:, AU:128], op=OP.mult)
                    nc.vector.tensor_reduce(out=qself[:, t:t + 1], in_=qs[:],
                                            axis=AX, op=OP.add)
                    nc.sync.dma_start(out=agKv[:, t, 0:AU], in_=qk[:, AU:128])
                    dV = work.tile([P, D], F32, tag="dV")
                    nc.scalar.mul(dV[:], h_stage[:, t * D:(t + 1) * D],
                                  dis_sb[:, t:t + 1])
                    nc.sync.dma_start(out=agKv[:, t, AU:128], in_=dV[:])

            zrow = pers.tile([1, 128], F32, tag="zrow")
            nc.vector.memset(zrow[:], 0.0)
            nc.sync.dma_start(out=ag_K_in[SLOTS - 1:SLOTS, :], in_=zrow[:])

            if not SIM_MODE:
                nc.gpsimd.collective_compute(
                    "AllGather", OP.bypass, replica_groups=RG,
                    ins=[ag_K_in.ap().opt()], outs=[K_full.ap().opt()])
            agh = ag_h_in[:, :].rearrange("(p t) d -> p t d", p=P)

            # ---------- phase 2: edge attention -> aff (gather order) ----------
            # fused with propagation step 0: KG cols 64:128 carry dis (.) V
            aff = pers.tile([P, Call], F32, tag="aff")

            for (col0, cols, tcol0, rinfo) in chunk_meta:
                sC = work.tile([P, CH], F32, tag="sC")   # chunk, tile-major
                mC = work.tile([P, CH], F32, tag="mC")
                hGb = cbuf.tile([P, CH, D], F32, tag="hGb")
                nc.sync.dma_start(out=mC[:, 0:cols],
                                  in_=mask_in[:, tcol0:tcol0 + cols])
                for (r, blocks, rcol0, rcols) in rinfo:
                    sub, cur, acc_w = [], [], 0
                    for blk in blocks:
                        if cur and acc_w + blk[2] > CH_ATT:
                            sub.append(cur)
                            cur, acc_w = [], 0
                        cur.append(blk)
                        acc_w += blk[2]
                    if cur:
                        sub.append(cur)
                    for blist in sub:
                        sc0 = rcol0 + blist[0][1]
                        sw = sum(b[2] for b in blist)
                        KG = work.tile([P, CH_ATT, 128], F32, tag="G")
                        nc.gpsimd.dma_gather(
                            out_ap=KG[:, 0:sw, :],
                            in_ap=K_full[r * RSZ:(r + 1) * RSZ, :],
                            idxs_ap=idx_res[:, 8 * sc0:8 * (sc0 + sw)],
                            num_idxs=128 * sw, num_idxs_reg=128 * sw,
                            elem_size=128, single_packet=False)
                        nc.scalar.copy(out=hGb[:, sc0 - col0:sc0 - col0 + sw, :],
                                       in_=KG[:, 0:sw, AU:128])
                        for (tt, loff, m) in blist:
                            lo = rcol0 + loff - sc0
                            tl = int(tcolbase[tt, r]) - tcol0
                            qb = Qn[:, tt * AU:(tt + 1) * AU]
                            qb = qb.unsqueeze(1).broadcast_to([P, m, AU])
                            nc.vector.tensor_tensor(
                                out=KG[:, lo:lo + m, 0:AU],
                                in0=KG[:, lo:lo + m, 0:AU], in1=qb,
                                op=OP.mult)
                            nc.vector.tensor_reduce(
                                out=sC[:, tl:tl + m],
                                in_=KG[:, lo:lo + m, 0:AU], axis=AX,
                                op=OP.add)
                # segment softmax + affinity per tile of this chunk
                tlist = [b[0] for b in rinfo[0][1]]
                for tt in tlist:
                    t0 = int(tcolbase[tt, 0]) - tcol0
                    mt = int(M[tt].sum())
                    seg = sC[:, t0:t0 + mt]
                    nc.vector.tensor_tensor(out=seg, in0=seg,
                                            in1=mC[:, t0:t0 + mt], op=OP.add)
                    mx = work.tile([P, 1], F32, tag="mx")
                    nc.vector.tensor_reduce(out=mx[:], in_=seg, axis=AX,
                                            op=OP.max)
                    nc.vector.tensor_tensor(out=mx[:], in0=mx[:],
                                            in1=qself[:, tt:tt + 1], op=OP.max)
                    nc.vector.tensor_scalar_mul(mx[:], mx[:], -1.0)
                    nc.scalar.activation(out=seg, in_=seg, func=AF.Exp,
                                         bias=mx[:])
                    selfe = work.tile([P, 1], F32, tag="selfe")
                    nc.scalar.activation(out=selfe[:], in_=qself[:, tt:tt + 1],
                                         func=AF.Exp, bias=mx[:])
                    zz = work.tile([P, 1], F32, tag="zz")
                    nc.vector.tensor_reduce(out=zz[:], in_=seg, axis=AX,
                                            op=OP.add)
                    nc.vector.tensor_tensor(out=zz[:], in0=zz[:],
                                            in1=selfe[:], op=OP.add)
                    nc.vector.reciprocal(zz[:], zz[:])
                    nc.vector.tensor_scalar_mul(seg, seg, zz[:])
                    nc.vector.tensor_tensor(out=gatself[:, tt:tt + 1],
                                            in0=selfe[:], in1=zz[:],
                                            op=OP.mult)
                    nc.vector.scalar_tensor_tensor(
                        out=selfa[:, tt:tt + 1], in0=gatself[:, tt:tt + 1],
                        scalar=c2_sb[:, tt:tt + 1], in1=c1_sb[:, tt:tt + 1],
                        op0=OP.mult, op1=OP.add)
                    nc.vector.tensor_tensor(out=selfa[:, tt:tt + 1],
                                            in0=selfa[:, tt:tt + 1],
                                            in1=dis_sb[:, tt:tt + 1],
                                            op=OP.mult)
                    nc.vector.scalar_tensor_tensor(
                        out=seg, in0=seg, scalar=c2_sb[:, tt:tt + 1],
                        in1=c1_sb[:, tt:tt + 1].broadcast_to([P, mt]),
                        op0=OP.mult, op1=OP.add)
                    for r in range(NRANGE):
                        tcr = int(tcolbase[tt, r]) - tcol0
                        gcr = int(gcolbase[tt, r])
                        m = int(M[tt, r])
                        nc.scalar.copy(out=aff[:, gcr:gcr + m],
                                       in_=sC[:, tcr:tcr + m])

                # fused propagation step 0 on the chunk's dis(.)V payload
                ab = aff[:, col0:col0 + cols]
                ab = ab.unsqueeze(2).broadcast_to([P, cols, D])
                nc.vector.tensor_tensor(out=hGb[:, 0:cols, :],
                                        in0=hGb[:, 0:cols, :], in1=ab,
                                        op=OP.mult)
                P4 = work.tile([P, TMAX, NRANGE, D], F32, tag="P4")
                for (r, blocks, rcol0, rcols) in rinfo:
                    for tl, (tt, loff, m) in enumerate(blocks):
                        lo = rcol0 - col0 + loff
                        nc.vector.tensor_reduce(
                            out=P4[:, tl, r, :],
                            in_=hGb[:, lo:lo + m, :].transpose([0, 2, 1]),
                            axis=AX, op=OP.add)
                nt_c = len(tlist)
                agg = work.tile([P, TMAX, D], F32, tag="agg")
                nc.vector.tensor_reduce(
                    out=agg[:, 0:nt_c, :],
                    in_=P4[:, 0:nt_c, :, :].transpose([0, 1, 3, 2]),
                    axis=AX, op=OP.add)
                hsc = work.tile([P, TMAX, D], F32, tag="hsc")
                tslb = work.tile([P, TMAX, D], F32, tag="tslb")
                t0c = tlist[0]
                hv = h_stage[:, t0c * D:(t0c + nt_c) * D].rearrange(
                    "p (t d) -> p t d", d=D)
                vv = Vp[:, t0c * D:(t0c + nt_c) * D].rearrange(
                    "p (t d) -> p t d", d=D)
                sb = selfa[:, t0c:t0c + nt_c].unsqueeze(2).broadcast_to(
                    [P, nt_c, D])
                dsb = dis_sb[:, t0c:t0c + nt_c].unsqueeze(2).broadcast_to(
                    [P, nt_c, D])
                nc.vector.tensor_tensor(out=tslb[:, 0:nt_c, :], in0=hv,
                                        in1=sb, op=OP.mult)
                nc.vector.tensor_tensor(out=tslb[:, 0:nt_c, :],
                                        in0=tslb[:, 0:nt_c, :], in1=vv,
                                        op=OP.add)
                nc.vector.tensor_tensor(out=hv, in0=agg[:, 0:nt_c, :],
                                        in1=tslb[:, 0:nt_c, :], op=OP.add)
                nc.vector.tensor_tensor(out=hsc[:, 0:nt_c, :], in0=hv,
                                        in1=dsb, op=OP.mult)
                nc.sync.dma_start(out=agh[:, t0c:t0c + nt_c, :],
                                  in_=hsc[:, 0:nt_c, :])

            if not SIM_MODE:
                nc.gpsimd.collective_compute(
                    "AllGather", OP.bypass, replica_groups=RG,
                    ins=[ag_h_in.ap().opt()], outs=[h_full.ap().opt()])

            # ---------- phase 3: propagation steps 1..T-1 ----------
            out_acc = pers.tile([P, NT * D], F32, tag="big1")  # reuses Qn slot
            nc.scalar.mul(out_acc[:], h_stage[:], hopw[:, 0:1])

            for step in range(1, T_LOOP):
                for (col0, cols, tcol0, rinfo) in chunk_meta:
                    G = work.tile([P, CH, D], F32, tag="G")
                    for (r, blocks, rcol0, rcols) in rinfo:
                        lo = rcol0 - col0
                        if "gather" in SKIP:
                            continue
                        nc.gpsimd.dma_gather(
                            out_ap=G[:, lo:lo + rcols, :],
                            in_ap=h_full[r * RSZ:(r + 1) * RSZ, :],
                            idxs_ap=idx_res[:, 8 * rcol0:8 * (rcol0 + rcols)],
                            num_idxs=128 * rcols, num_idxs_reg=128 * rcols,
                            elem_size=D, single_packet=False)
                    ab = aff[:, col0:col0 + cols]
                    ab = ab.unsqueeze(2).broadcast_to([P, cols, D])
                    if "mult" not in SKIP:
                        nc.vector.tensor_tensor(out=G[:, 0:cols, :],
                                                in0=G[:, 0:cols, :], in1=ab,
                                                op=OP.mult)
                    P4 = work.tile([P, TMAX, NRANGE, D], F32, tag="P4")
                    tlist = [b[0] for b in rinfo[0][1]]
                    for (r, blocks, rcol0, rcols) in rinfo:
                        for tl, (tt, loff, m) in enumerate(blocks):
                            lo = rcol0 - col0 + loff
                            if "reduce" in SKIP:
                                continue
                            nc.vector.tensor_reduce(
                                out=P4[:, tl, r, :],
                                in_=G[:, lo:lo + m, :].transpose([0, 2, 1]),
                                axis=AX, op=OP.add)
                    nt_c = len(tlist)
                    agg = work.tile([P, TMAX, D], F32, tag="agg")
                    nc.vector.tensor_reduce(
                        out=agg[:, 0:nt_c, :],
                        in_=P4[:, 0:nt_c, :, :].transpose([0, 1, 3, 2]),
                        axis=AX, op=OP.add)
                    hsc = work.tile([P, TMAX, D], F32, tag="hsc")
                    tslb = work.tile([P, TMAX, D], F32, tag="tslb")
                    t0c = tlist[0]
                    hv = h_stage[:, t0c * D:(t0c + nt_c) * D].rearrange(
                        "p (t d) -> p t d", d=D)
                    vv = Vp[:, t0c * D:(t0c + nt_c) * D].rearrange(
                        "p (t d) -> p t d", d=D)
                    ov = out_acc[:, t0c * D:(t0c + nt_c) * D].rearrange(
                        "p (t d) -> p t d", d=D)
                    # h (.) selfa on the (mostly idle) Activation engine
                    for tl, tt in enumerate(tlist):
                        nc.scalar.mul(tslb[:, tl, :],
                                      h_stage[:, tt * D:(tt + 1) * D],
                                      selfa[:, tt:tt + 1])
                    nc.vector.tensor_tensor(out=tslb[:, 0:nt_c, :],
                                            in0=tslb[:, 0:nt_c, :], in1=vv,
                                            op=OP.add)
                    nc.vector.tensor_tensor(out=hv, in0=agg[:, 0:nt_c, :],
                                            in1=tslb[:, 0:nt_c, :], op=OP.add)
                    nc.vector.scalar_tensor_tensor(
                        out=ov, in0=hv, scalar=hopw[:, step:step + 1],
                        in1=ov, op0=OP.mult, op1=OP.add)
                    if step < T_LOOP - 1:
                        for tl, tt in enumerate(tlist):
                            nc.scalar.mul(hsc[:, tl, :],
                                          h_stage[:, tt * D:(tt + 1) * D],
                                          dis_sb[:, tt:tt + 1])
                        nc.sync.dma_start(
                            out=agh[:, t0c:t0c + nt_c, :],
                            in_=hsc[:, 0:nt_c, :])
                if step < T_LOOP - 1:
                    if not SIM_MODE:
                        nc.gpsimd.collective_compute(
                            "AllGather", OP.bypass, replica_groups=RG,
                            ins=[ag_h_in.ap().opt()], outs=[h_full.ap().opt()])

            nc.sync.dma_start(out=out_dram[:, :], in_=out_acc[:])

    nc.compile()
    return nc


def kernel(**inputs):
    x = np.asarray(inputs["x"], np.float32)
    edge_index = np.asarray(inputs["edge_index"])
    pp = _preprocess(edge_index)
    nc = _build_kernel(pp)

    nodes = pp["nodes_by_rank"]
    in_maps = []
    for c in range(NCORES):
        vs = nodes[c]
        xsh = np.zeros((SLOTS, F), np.float32)
        real = vs >= 0
        xsh[np.where(real)[0]] = x[vs[real]]
        in_maps.append({
            "xT": np.ascontiguousarray(xsh.T),
            "idxw": np.ascontiguousarray(pp["idxw"][c]),
            "maskNeg": np.ascontiguousarray(pp["maskNeg"][c]),
            "deg_ell": np.ascontiguousarray(pp["deg_ell"][c]),
            "W1": np.asarray(inputs["W1"], np.float32),
            "W2": np.asarray(inputs["W2"], np.float32),
            "Wq": np.asarray(inputs["Wq"], np.float32),
            "Wk": np.asarray(inputs["Wk"], np.float32),
            "b1": np.asarray(inputs["b1"], np.float32),
            "bq": np.asarray(inputs["bq"], np.float32),
            "bk": np.asarray(inputs["bk"], np.float32),
            "b2": np.asarray(inputs["b2"], np.float32),
            "att": np.asarray(inputs["att_logits"], np.float32),
        })

    global LAST_RESULTS
    res = bass_utils.run_bass_kernel_spmd(nc, in_maps,
                                          core_ids=list(range(NCORES)),
                                          trace=TRACE)
    LAST_RESULTS = res

    out = np.zeros((N, D), np.float32)
    for c in range(NCORES):
        oc = np.asarray(res.results[c]["out"]).reshape(P, NT, D)
        vs = nodes[c]
        real = np.where(vs >= 0)[0]
        out[vs[real]] = oc[real % P, real // P]
    return out



# revision 37
# speedup vs baseline: 1.0023x; 1.0023x over previous
"""CAPGNN message-passing kernel for 8 Trainium2 NeuronCores.

Sharding (per hint): nodes partitioned across the 8 cores; per core the
destinations are degree-sorted into 98 tiles of 128 (one dest per SBUF
partition) with non-self edges in an ELL layout along the free axis, split
into 4 source ranges of 25088 rows so dma_gather's int16 indices can address
the replicated tables. Self loops are excluded from the edge stream and
folded into the per-step update as a per-dest scalar (selfa). The gathered
h table holds dis (.) h (deg^-1/2 pre-scaled), so the edge affinity drops
its dis[col] factor; (1-ALPHA) is folded into the affinity as well. The
attention pass gathers 512B rows [K | dis (.) V] and its second half feeds
propagation step 0 directly (one fewer gather pass). Gather index tables are
SBUF-resident across all steps. Each remaining step gathers h[col] with
batched dma_gather, multiplies by the resident affinity, segment-reduces on
DVE, updates h, and AllGathers the re-scaled shard. Dense weights are
replicated.
"""
import numpy as np

from concourse import bacc, mybir, tile, library_config
from concourse import bass_utils

F32 = mybir.dt.float32
I16 = mybir.dt.int16

N = 100000
F = 128
D = 64
AU = 64
T = 10
ALPHA = 0.1
BETA = 0.3
NCORES = 8
NSHARD = 12500
SLOTS = 12544          # 98 * 128 (44 phantom slots)
NT = 98
P = 128
NRANGE = 4
RSZ = 2 * SLOTS        # 25088 rows per gather range (int16-addressable)
PAD_LOCAL = SLOTS + 12543  # zero pad row: odd core of the pair, slot 12543
CH = 64                # max ELL columns per propagation chunk
CH_ATT = 32            # max cols per attention sub-call (KG rows are 512B)
NEG = -1.0e30
TRACE = False
LAST_RESULTS = None
SIM_MODE = False
SKIP = set()
T_LOOP = T


def _preprocess(edge_index):
    row = edge_index[0].astype(np.int64)
    col = edge_index[1].astype(np.int64)
    nonself = row != col
    row = row[nonself]
    col = col[nonself]
    # deg matches the reference (self loop added there)
    deg = np.bincount(row, minlength=N).astype(np.int64) + 1

    # greedy node->pair assignment balancing every dest's per-range counts
    order_c = np.argsort(col, kind="stable")
    dests_by_src = row[order_c]
    src_off = np.zeros(N + 1, np.int64)
    np.cumsum(np.bincount(col, minlength=N), out=src_off[1:])
    quarter = ((deg - 1 + NRANGE - 1) // NRANGE).astype(np.int32)
    # exponential excess penalty: flattening per-dest per-range counts
    WT = np.zeros(24, np.int64)
    for i in range(24):
        e = i - 8
        WT[i] = 0 if e < 0 else 4 ** min(e, 8)
    cnt = np.zeros((N, NRANGE), np.int32)
    pair_of = np.full(N, -1, np.int32)
    fill = np.zeros(NRANGE, np.int64)
    cap = 2 * NSHARD
    out_deg = np.diff(src_off)
    visit = np.argsort(-out_deg, kind="stable")
    for v in visit:
        ds = dests_by_src[src_off[v]:src_off[v + 1]]
        if ds.size:
            exc = np.clip(cnt[ds] + 1 - quarter[ds, None] + 8, 0, 23)
            over = WT[exc].sum(0)
        else:
            over = np.zeros(NRANGE, np.int64)
        score = over * (cap + 1) + fill
        score = score + np.where(fill >= cap, np.int64(1) << 50, 0)
        r = int(np.argmin(score))
        pair_of[v] = r
        fill[r] += 1
        if ds.size:
            cnt[ds, r] += 1

    core_of = np.full(N, -1, np.int32)
    for r in range(NRANGE):
        vs = np.where(pair_of == r)[0]
        vs = vs[np.argsort(-deg[vs], kind="stable")]
        core_of[vs[0::2]] = 2 * r
        core_of[vs[1::2]] = 2 * r + 1

    # order dests within each core so tiles group similar count vectors
    # (minimizes per-tile per-range maxima -> less ELL padding)
    cnts = np.bincount(row * NRANGE + pair_of[col],
                       minlength=N * NRANGE).reshape(N, NRANGE)
    mx = cnts.max(1)
    am = cnts.argmax(1)
    ghat = np.zeros(N, np.int64)
    tau_of = np.zeros(N, np.int64)
    p_of = np.zeros(N, np.int64)
    nodes_by_rank = np.full((NCORES, SLOTS), -1, np.int64)
    for c in range(NCORES):
        vs = np.where(core_of == c)[0]
        vs = vs[np.lexsort((-cnts[vs, 3], -cnts[vs, 2], -cnts[vs, 1],
                            -cnts[vs, 0], am[vs], -mx[vs]))]
        k = np.arange(vs.size)
        tau_of[vs] = k // P
        p_of[vs] = k % P
        ghat[vs] = c * SLOTS + p_of[vs] * NT + tau_of[vs]
        nodes_by_rank[c, :vs.size] = vs

    gcol = ghat[col]
    e_core = core_of[row]
    e_tau = tau_of[row]
    e_p = p_of[row]
    e_r = gcol // RSZ
    e_lcol = (gcol % RSZ).astype(np.int64)

    key = ((e_core * NT + e_tau) * P + e_p) * NRANGE + e_r
    nkeys = NCORES * NT * P * NRANGE
    counts = np.bincount(key, minlength=nkeys)
    starts = np.zeros(nkeys + 1, np.int64)
    np.cumsum(counts, out=starts[1:])
    eo = np.argsort(key, kind="stable")
    j_sorted = np.arange(key.size, dtype=np.int64) - starts[key[eo]]
    j_of = np.zeros(key.size, np.int64)
    j_of[eo] = j_sorted

    cnt4 = counts.reshape(NCORES, NT, P, NRANGE)
    M = np.maximum(cnt4.max(axis=(0, 2)), 1).astype(np.int64)  # [NT, NRANGE]

    tcolbase = np.zeros((NT, NRANGE), np.int64)
    acc = 0
    for t in range(NT):
        for r in range(NRANGE):
            tcolbase[t, r] = acc
            acc += int(M[t, r])
    Call = int(acc)

    chunks = []
    cur, cur_w = [], 0
    for t in range(NT):
        w = int(M[t].sum())
        if cur and cur_w + w > CH:
            chunks.append(cur)
            cur, cur_w = [], 0
        cur.append(t)
        cur_w += w
    if cur:
        chunks.append(cur)

    gcolbase = np.zeros((NT, NRANGE), np.int64)
    acc = 0
    chunk_meta = []  # (col0, cols, tcol0, [(r, [(tau, loff_in_r, M)], rcol0, rcols)])
    for ch in chunks:
        col0 = acc
        rinfo = []
        for r in range(NRANGE):
            rcol0 = acc
            blocks = []
            for t in ch:
                gcolbase[t, r] = acc
                blocks.append((t, acc - rcol0, int(M[t, r])))
                acc += int(M[t, r])
            rinfo.append((r, blocks, rcol0, acc - rcol0))
        chunk_meta.append((col0, acc - col0, int(tcolbase[ch[0], 0]), rinfo))
    assert acc == Call

    col16 = np.full((NCORES, P, Call), PAD_LOCAL, np.int16)
    maskNeg = np.full((NCORES, P, Call), NEG, np.float32)
    e_gc = gcolbase[e_tau, e_r] + j_of
    e_tc = tcolbase[e_tau, e_r] + j_of
    col16[e_core, e_p, e_gc] = e_lcol.astype(np.int16)
    maskNeg[e_core, e_p, e_tc] = 0.0
    # phantom rows (tile 97, partitions 84.. on every core): unmask their
    # first r0 pad column so softmax Z = 1 instead of 0 (no NaN).
    maskNeg[:, 84:, tcolbase[NT - 1, 0]] = 0.0

    idxw = np.zeros((NCORES, P, 8 * Call), np.int16)
    for c in range(NCORES):
        for (col0, cols, tcol0, rinfo) in chunk_meta:
            for (r, blocks, rcol0, rcols) in rinfo:
                A = col16[c][:, rcol0:rcol0 + rcols]
                flat = A.T.reshape(-1)
                w16 = flat.reshape(-1, 16).T
                idxw[c][:, 8 * rcol0:8 * (rcol0 + rcols)] = np.tile(w16, (8, 1))

    deg_ell = np.ones((NCORES, P, NT), np.float32)
    for c in range(NCORES):
        vs = nodes_by_rank[c]
        real = vs >= 0
        k = np.arange(SLOTS)
        deg_ell[c, k[real] % P, k[real] // P] = deg[vs[real]].astype(np.float32)

    return dict(deg=deg, core_of=core_of, nodes_by_rank=nodes_by_rank,
                M=M, Call=Call, chunks=chunks, chunk_meta=chunk_meta,
                tcolbase=tcolbase, gcolbase=gcolbase,
                col16=col16, maskNeg=maskNeg, idxw=idxw, deg_ell=deg_ell)


def _build_kernel(pp):
    Call = pp["Call"]
    chunk_meta = pp["chunk_meta"]
    tcolbase = pp["tcolbase"]
    gcolbase = pp["gcolbase"]
    M = pp["M"]
    TMAX = max(len(ch) for ch in pp["chunks"])

    nc = bacc.Bacc("TRN2", target_bir_lowering=False, debug=False,
                   num_devices=1 if SIM_MODE else NCORES)

    xT_in = nc.dram_tensor("xT", [P, SLOTS], F32, kind="ExternalInput")
    idx_in = nc.dram_tensor("idxw", [P, 8 * Call], I16, kind="ExternalInput")
    mask_in = nc.dram_tensor("maskNeg", [P, Call], F32, kind="ExternalInput")
    deg_in = nc.dram_tensor("deg_ell", [P, NT], F32, kind="ExternalInput")
    W1_in = nc.dram_tensor("W1", [F, 128], F32, kind="ExternalInput")
    W2_in = nc.dram_tensor("W2", [128, D], F32, kind="ExternalInput")
    Wq_in = nc.dram_tensor("Wq", [F, AU], F32, kind="ExternalInput")
    Wk_in = nc.dram_tensor("Wk", [F, AU], F32, kind="ExternalInput")
    b1_in = nc.dram_tensor("b1", [128], F32, kind="ExternalInput")
    bq_in = nc.dram_tensor("bq", [AU], F32, kind="ExternalInput")
    bk_in = nc.dram_tensor("bk", [AU], F32, kind="ExternalInput")
    b2_in = nc.dram_tensor("b2", [D], F32, kind="ExternalInput")
    att_in = nc.dram_tensor("att", [T], F32, kind="ExternalInput")
    out_dram = nc.dram_tensor("out", [P, NT * D], F32, kind="ExternalOutput")

    ag_h_in = nc.dram_tensor("ag_h_in", [SLOTS, D], F32)
    ag_K_in = nc.dram_tensor("ag_K_in", [SLOTS, 128], F32)
    # NOTE: dma_gather from Shared scratchpad faults the device; keep the
    # gather tables in Local scratchpad (AllGather to Local is supported).
    h_full = nc.dram_tensor("h_full", [NCORES * SLOTS, D], F32)
    K_full = nc.dram_tensor("K_full", [NCORES * SLOTS, 128], F32)
    RG = [list(range(NCORES))]

    AX = mybir.AxisListType.X
    OP = mybir.AluOpType
    AF = mybir.ActivationFunctionType

    with tile.TileContext(nc) as tc:
        nc.gpsimd.load_library(library_config.mlp)
        with tc.tile_pool(name="pers", bufs=1) as pers, \
             tc.tile_pool(name="work", bufs=2) as work, \
             tc.tile_pool(name="cbuf", bufs=2) as cbuf, \
             tc.tile_pool(name="psum", bufs=2, space="PSUM") as psum:

            # ---------- phase 0: weights / biases / broadcasts ----------
            W1_sb = pers.tile([F, 128], F32, tag="W1")
            W2_sb = pers.tile([128, D], F32, tag="W2")
            WqWk = pers.tile([F, 128], F32, tag="WqWk")
            b1_col = pers.tile([P, 1], F32, tag="b1")
            nc.sync.dma_start(out=W1_sb[:], in_=W1_in[:, :])
            nc.sync.dma_start(out=W2_sb[:], in_=W2_in[:, :])
            nc.sync.dma_start(out=WqWk[:, 0:AU], in_=Wq_in[:, :])
            nc.sync.dma_start(out=WqWk[:, AU:128], in_=Wk_in[:, :])
            nc.vector.tensor_scalar_mul(WqWk[:, 0:AU], WqWk[:, 0:AU], 0.125)
            nc.sync.dma_start(out=b1_col[:], in_=b1_in[:].unsqueeze(1))

            brow = pers.tile([1, 330], F32, tag="brow")
            nc.sync.dma_start(out=brow[0:1, 0:64], in_=bq_in[:].unsqueeze(0))
            nc.sync.dma_start(out=brow[0:1, 64:128], in_=bk_in[:].unsqueeze(0))
            nc.sync.dma_start(out=brow[0:1, 128:192], in_=b2_in[:].unsqueeze(0))
            nc.sync.dma_start(out=brow[0:1, 192:256], in_=b2_in[:].unsqueeze(0))
            nc.sync.dma_start(out=brow[0:1, 256:266], in_=att_in[:].unsqueeze(0))
            nc.vector.tensor_scalar_mul(brow[0:1, 0:64], brow[0:1, 0:64], 0.125)
            nc.vector.tensor_scalar_mul(brow[0:1, 192:256],
                                        brow[0:1, 192:256], ALPHA)
            bcast = pers.tile([P, 330], F32, tag="bcast")
            nc.gpsimd.partition_broadcast(bcast[:], brow[:])
            qkb = bcast[:, 0:128]
            b2b = bcast[:, 128:192]
            b2bA = bcast[:, 192:256]
            attb = bcast[:, 256:266]

            hopw = pers.tile([P, T], F32, tag="hopw")
            sc1 = pers.tile([P, 1], F32, tag="sc1")
            nc.scalar.activation(out=hopw[:], in_=attb, func=AF.Lrelu,
                                 alpha=0.2)
            nc.vector.tensor_reduce(out=sc1[:], in_=hopw[:], axis=AX,
                                    op=OP.max)
            nc.vector.tensor_scalar_mul(sc1[:], sc1[:], -1.0)
            nc.scalar.activation(out=hopw[:], in_=hopw[:], func=AF.Exp,
                                 bias=sc1[:])
            nc.vector.tensor_reduce(out=sc1[:], in_=hopw[:], axis=AX,
                                    op=OP.add)
            nc.vector.tensor_scalar_mul(sc1[:], sc1[:], 1.0 + 1e-8)
            nc.vector.reciprocal(sc1[:], sc1[:])
            nc.vector.tensor_scalar_mul(hopw[:], hopw[:], sc1[:])

            deg_sb = pers.tile([P, NT], F32, tag="deg")
            ds_sb = pers.tile([P, NT], F32, tag="ds")
            dis_sb = pers.tile([P, NT], F32, tag="dis")
            c1_sb = pers.tile([P, NT], F32, tag="c1")
            c2_sb = pers.tile([P, NT], F32, tag="c2")
            nc.sync.dma_start(out=deg_sb[:], in_=deg_in[:, :])
            nc.scalar.sqrt(out=ds_sb[:], in_=deg_sb[:])
            nc.vector.reciprocal(dis_sb[:], ds_sb[:])
            # fold (1 - ALPHA) into the edge affinity so the step update is
            # h = agg + (selfa (.) h_old + Vp)
            nc.vector.tensor_scalar_mul(c1_sb[:], dis_sb[:],
                                        BETA * (1.0 - ALPHA))
            nc.vector.tensor_scalar_mul(c2_sb[:], ds_sb[:],
                                        (1.0 - BETA) * (1.0 - ALPHA))

            # resident gather-index table (used by phase 2 and phase 3)
            idx_res = pers.tile([P, 8 * Call], I16, tag="idxres")
            nc.sync.dma_start(out=idx_res[:], in_=idx_in[:, :])

            # ---------- phase 1: dense V / Q / K ----------
            Vp = pers.tile([P, NT * D], F32, tag="Vp")
            h_stage = pers.tile([P, NT * D], F32, tag="hstage")
            Qn = pers.tile([P, NT * AU], F32, tag="big1")  # shared w/ out_acc
            qself = pers.tile([P, NT], F32, tag="qself")
            gatself = pers.tile([P, NT], F32, tag="gatself")
            selfa = pers.tile([P, NT], F32, tag="selfa")
            agKv = ag_K_in[:, :].rearrange("(p t) d -> p t d", p=P)

            for g0 in range(0, NT, 2):
                gt = min(2, NT - g0)
                xc = work.tile([P, 256], F32, tag="xc")
                nc.sync.dma_start(out=xc[:, 0:gt * P],
                                  in_=xT_in[:, g0 * P:(g0 + gt) * P])
                a_ps = psum.tile([P, 256], F32, tag="aps")
                nc.tensor.matmul(out=a_ps[:, 0:gt * P], lhsT=W1_sb[:],
                                 rhs=xc[:, 0:gt * P], start=True, stop=True)
                a1 = work.tile([P, 256], F32, tag="a1")
                nc.scalar.activation(out=a1[:, 0:gt * P], in_=a_ps[:, 0:gt * P],
                                     func=AF.Relu, bias=b1_col[:])
                for s in range(gt):
                    t = g0 + s
                    v_ps = psum.tile([P, D], F32, tag="vps")
                    nc.tensor.matmul(out=v_ps[:],
                                     lhsT=a1[:, s * P:(s + 1) * P],
                                     rhs=W2_sb[:], start=True, stop=True)
                    nc.vector.scalar_tensor_tensor(
                        out=h_stage[:, t * D:(t + 1) * D], in0=v_ps[:],
                        scalar=1.0, in1=b2b, op0=OP.mult, op1=OP.add)
                    nc.vector.scalar_tensor_tensor(
                        out=Vp[:, t * D:(t + 1) * D], in0=v_ps[:],
                        scalar=ALPHA, in1=b2bA, op0=OP.mult, op1=OP.add)
                    qk_ps = psum.tile([P, 128], F32, tag="qkps")
                    nc.tensor.matmul(out=qk_ps[:],
                                     lhsT=xc[:, s * P:(s + 1) * P],
                                     rhs=WqWk[:], start=True, stop=True)
                    qk = work.tile([P, 128], F32, tag="qk")
                    nc.vector.tensor_tensor(out=qk[:], in0=qk_ps[:], in1=qkb,
                                            op=OP.add)
                    nc.scalar.activation(out=qk[:], in_=qk[:], func=AF.Relu)
                    nc.vector.tensor_copy(out=Qn[:, t * AU:(t + 1) * AU],
                                          in_=qk[:, 0:AU])
                    qs = work.tile([P, AU], F32, tag="qs")
                    nc.vector.tensor_tensor(out=qs[:], in0=qk[:, 0:AU],
                                            in1=qk[:, AU:128], op=OP.mult)
                    nc.vector.tensor_reduce(out=qself[:, t:t + 1], in_=qs[:],
                                            axis=AX, op=OP.add)
                    nc.sync.dma_start(out=agKv[:, t, 0:AU], in_=qk[:, AU:128])
                    dV = work.tile([P, D], F32, tag="dV")
                    nc.scalar.mul(dV[:], h_stage[:, t * D:(t + 1) * D],
                                  dis_sb[:, t:t + 1])
                    nc.sync.dma_start(out=agKv[:, t, AU:128], in_=dV[:])

            zrow = pers.tile([1, 128], F32, tag="zrow")
            nc.vector.memset(zrow[:], 0.0)
            nc.sync.dma_start(out=ag_K_in[SLOTS - 1:SLOTS, :], in_=zrow[:])

            if not SIM_MODE:
                nc.gpsimd.collective_compute(
                    "AllGather", OP.bypass, replica_groups=RG,
                    ins=[ag_K_in.ap().opt()], outs=[K_full.ap().opt()])
            agh = ag_h_in[:, :].rearrange("(p t) d -> p t d", p=P)

            # ---------- phase 2: edge attention -> aff (gather order) ----------
            # fused with propagation step 0: KG cols 64:128 carry dis (.) V
            aff = pers.tile([P, Call], F32, tag="aff")

            for (col0, cols, tcol0, rinfo) in chunk_meta:
                sC = work.tile([P, CH], F32, tag="sC")   # chunk, tile-major
                mC = work.tile([P, CH], F32, tag="mC")
                hGb = cbuf.tile([P, CH, D], F32, tag="hGb")
                nc.sync.dma_start(out=mC[:, 0:cols],
                                  in_=mask_in[:, tcol0:tcol0 + cols])
                for (r, blocks, rcol0, rcols) in rinfo:
                    sub, cur, acc_w = [], [], 0
                    for blk in blocks:
                        if cur and acc_w + blk[2] > CH_ATT:
                            sub.append(cur)
                            cur, acc_w = [], 0
                        cur.append(blk)
                        acc_w += blk[2]
                    if cur:
                        sub.append(cur)
                    for blist in sub:
                        sc0 = rcol0 + blist[0][1]
                        sw = sum(b[2] for b in blist)
                        KG = work.tile([P, CH_ATT, 128], F32, tag="G")
                        nc.gpsimd.dma_gather(
                            out_ap=KG[:, 0:sw, :],
                            in_ap=K_full[r * RSZ:(r + 1) * RSZ, :],
                            idxs_ap=idx_res[:, 8 * sc0:8 * (sc0 + sw)],
                            num_idxs=128 * sw, num_idxs_reg=128 * sw,
                            elem_size=128, single_packet=False)
                        nc.scalar.copy(out=hGb[:, sc0 - col0:sc0 - col0 + sw, :],
                                       in_=KG[:, 0:sw, AU:128])
                        for (tt, loff, m) in blist:
                            lo = rcol0 + loff - sc0
                            tl = int(tcolbase[tt, r]) - tcol0
                            qb = Qn[:, tt * AU:(tt + 1) * AU]
                            qb = qb.unsqueeze(1).broadcast_to([P, m, AU])
                            nc.vector.tensor_tensor(
                                out=KG[:, lo:lo + m, 0:AU],
                                in0=KG[:, lo:lo + m, 0:AU], in1=qb,
                                op=OP.mult)
                            nc.vector.tensor_reduce(
                                out=sC[:, tl:tl + m],
                                in_=KG[:, lo:lo + m, 0:AU], axis=AX,
                                op=OP.add)
                # segment softmax + affinity per tile of this chunk
                tlist = [b[0] for b in rinfo[0][1]]
                nc.vector.tensor_tensor(out=sC[:, 0:cols], in0=sC[:, 0:cols],
                                        in1=mC[:, 0:cols], op=OP.add)
                for tt in tlist:
                    t0 = int(tcolbase[tt, 0]) - tcol0
                    mt = int(M[tt].sum())
                    seg = sC[:, t0:t0 + mt]
                    mx = work.tile([P, 1], F32, tag="mx")
                    nc.vector.tensor_reduce(out=mx[:], in_=seg, axis=AX,
                                            op=OP.max)
                    nc.vector.tensor_tensor(out=mx[:], in0=mx[:],
                                            in1=qself[:, tt:tt + 1], op=OP.max)
                    nc.vector.tensor_scalar_mul(mx[:], mx[:], -1.0)
                    nc.scalar.activation(out=seg, in_=seg, func=AF.Exp,
                                         bias=mx[:])
                    selfe = work.tile([P, 1], F32, tag="selfe")
                    nc.scalar.activation(out=selfe[:], in_=qself[:, tt:tt + 1],
                                         func=AF.Exp, bias=mx[:])
                    zz = work.tile([P, 1], F32, tag="zz")
                    nc.vector.tensor_reduce(out=zz[:], in_=seg, axis=AX,
                                            op=OP.add)
                    nc.vector.tensor_tensor(out=zz[:], in0=zz[:],
                                            in1=selfe[:], op=OP.add)
                    nc.vector.reciprocal(zz[:], zz[:])
                    nc.vector.tensor_scalar_mul(seg, seg, zz[:])
                    nc.vector.tensor_tensor(out=gatself[:, tt:tt + 1],
                                            in0=selfe[:], in1=zz[:],
                                            op=OP.mult)
                    nc.vector.scalar_tensor_tensor(
                        out=selfa[:, tt:tt + 1], in0=gatself[:, tt:tt + 1],
                        scalar=c2_sb[:, tt:tt + 1], in1=c1_sb[:, tt:tt + 1],
                        op0=OP.mult, op1=OP.add)
                    nc.vector.tensor_tensor(out=selfa[:, tt:tt + 1],
                                            in0=selfa[:, tt:tt + 1],
                                            in1=dis_sb[:, tt:tt + 1],
                                            op=OP.mult)
                    nc.vector.scalar_tensor_tensor(
                        out=seg, in0=seg, scalar=c2_sb[:, tt:tt + 1],
                        in1=c1_sb[:, tt:tt + 1].broadcast_to([P, mt]),
                        op0=OP.mult, op1=OP.add)
                    for r in range(NRANGE):
                        tcr = int(tcolbase[tt, r]) - tcol0
                        gcr = int(gcolbase[tt, r])
                        m = int(M[tt, r])
                        nc.scalar.copy(out=aff[:, gcr:gcr + m],
                                       in_=sC[:, tcr:tcr + m])

                # fused propagation step 0 on the chunk's dis(.)V payload
                ab = aff[:, col0:col0 + cols]
                ab = ab.unsqueeze(2).broadcast_to([P, cols, D])
                nc.vector.tensor_tensor(out=hGb[:, 0:cols, :],
                                        in0=hGb[:, 0:cols, :], in1=ab,
                                        op=OP.mult)
                P4 = work.tile([P, TMAX, NRANGE, D], F32, tag="P4")
                for (r, blocks, rcol0, rcols) in rinfo:
                    for tl, (tt, loff, m) in enumerate(blocks):
                        lo = rcol0 - col0 + loff
                        nc.vector.tensor_reduce(
                            out=P4[:, tl, r, :],
                            in_=hGb[:, lo:lo + m, :].transpose([0, 2, 1]),
                            axis=AX, op=OP.add)
                nt_c = len(tlist)
                agg = work.tile([P, TMAX, D], F32, tag="agg")
                nc.vector.tensor_reduce(
                    out=agg[:, 0:nt_c, :],
                    in_=P4[:, 0:nt_c, :, :].transpose([0, 1, 3, 2]),
                    axis=AX, op=OP.add)
                hsc = work.tile([P, TMAX, D], F32, tag="hsc")
                tslb = work.tile([P, TMAX, D], F32, tag="tslb")
                t0c = tlist[0]
                hv = h_stage[:, t0c * D:(t0c + nt_c) * D].rearrange(
                    "p (t d) -> p t d", d=D)
                vv = Vp[:, t0c * D:(t0c + nt_c) * D].rearrange(
                    "p (t d) -> p t d", d=D)
                sb = selfa[:, t0c:t0c + nt_c].unsqueeze(2).broadcast_to(
                    [P, nt_c, D])
                dsb = dis_sb[:, t0c:t0c + nt_c].unsqueeze(2).broadcast_to(
                    [P, nt_c, D])
                nc.vector.tensor_tensor(out=tslb[:, 0:nt_c, :], in0=hv,
                                        in1=sb, op=OP.mult)
                nc.vector.tensor_tensor(out=tslb[:, 0:nt_c, :],
                                        in0=tslb[:, 0:nt_c, :], in1=vv,
                                        op=OP.add)
                nc.vector.tensor_tensor(out=hv, in0=agg[:, 0:nt_c, :],
                                        in1=tslb[:, 0:nt_c, :], op=OP.add)
                nc.vector.tensor_tensor(out=hsc[:, 0:nt_c, :], in0=hv,
                                        in1=dsb, op=OP.mult)
                nc.sync.dma_start(out=agh[:, t0c:t0c + nt_c, :],
                                  in_=hsc[:, 0:nt_c, :])

            if not SIM_MODE:
                nc.gpsimd.collective_compute(
                    "AllGather", OP.bypass, replica_groups=RG,
                    ins=[ag_h_in.ap().opt()], outs=[h_full.ap().opt()])

            # ---------- phase 3: propagation steps 1..T-1 ----------
            out_acc = pers.tile([P, NT * D], F32, tag="big1")  # reuses Qn slot
            nc.scalar.mul(out_acc[:], h_stage[:], hopw[:, 0:1])

            for step in range(1, T_LOOP):
                for (col0, cols, tcol0, rinfo) in chunk_meta:
                    G = work.tile([P, CH, D], F32, tag="G")
                    for (r, blocks, rcol0, rcols) in rinfo:
                        lo = rcol0 - col0
                        if "gather" in SKIP:
                            continue
                        nc.gpsimd.dma_gather(
                            out_ap=G[:, lo:lo + rcols, :],
                            in_ap=h_full[r * RSZ:(r + 1) * RSZ, :],
                            idxs_ap=idx_res[:, 8 * rcol0:8 * (rcol0 + rcols)],
                            num_idxs=128 * rcols, num_idxs_reg=128 * rcols,
                            elem_size=D, single_packet=False)
                    ab = aff[:, col0:col0 + cols]
                    ab = ab.unsqueeze(2).broadcast_to([P, cols, D])
                    if "mult" not in SKIP:
                        nc.vector.tensor_tensor(out=G[:, 0:cols, :],
                                                in0=G[:, 0:cols, :], in1=ab,
                                                op=OP.mult)
                    P4 = work.tile([P, TMAX, NRANGE, D], F32, tag="P4")
                    tlist = [b[0] for b in rinfo[0][1]]
                    for (r, blocks, rcol0, rcols) in rinfo:
                        for tl, (tt, loff, m) in enumerate(blocks):
                            lo = rcol0 - col0 + loff
                            if "reduce" in SKIP:
                                continue
                            nc.vector.tensor_reduce(
                                out=P4[:, tl, r, :],
                                in_=G[:, lo:lo + m, :].transpose([0, 2, 1]),
                                axis=AX, op=OP.add)
                    nt_c = len(tlist)
                    agg = work.tile([P, TMAX, D], F32, tag="agg")
                    nc.vector.tensor_reduce(
                        out=agg[:, 0:nt_c, :],
                        in_=P4[:, 0:nt_c, :, :].transpose([0, 1, 3, 2]),
                        axis=AX, op=OP.add)
                    hsc = work.tile([P, TMAX, D], F32, tag="hsc")
                    tslb = work.tile([P, TMAX, D], F32, tag="tslb")
                    t0c = tlist[0]
                    hv = h_stage[:, t0c * D:(t0c + nt_c) * D].rearrange(
                        "p (t d) -> p t d", d=D)
                    vv = Vp[:, t0c * D:(t0c + nt_c) * D].rearrange(
                        "p (t d) -> p t d", d=D)
                    ov = out_acc[:, t0c * D:(t0c + nt_c) * D].rearrange(
                        "p (t d) -> p t d", d=D)
                    # h (.) selfa on the (mostly idle) Activation engine
                    for tl, tt in enumerate(tlist):
                        nc.scalar.mul(tslb[:, tl, :],
                                      h_stage[:, tt * D:(tt + 1) * D],
                                      selfa[:, tt:tt + 1])
                    nc.vector.tensor_tensor(out=tslb[:, 0:nt_c, :],
                                            in0=tslb[:, 0:nt_c, :], in1=vv,
                                            op=OP.add)
                    nc.vector.tensor_tensor(out=hv, in0=agg[:, 0:nt_c, :],
                                            in1=tslb[:, 0:nt_c, :], op=OP.add)
                    nc.vector.scalar_tensor_tensor(
                        out=ov, in0=hv, scalar=hopw[:, step:step + 1],
                        in1=ov, op0=OP.mult, op1=OP.add)
                    if step < T_LOOP - 1:
                        for tl, tt in enumerate(tlist):
                            nc.scalar.mul(hsc[:, tl, :],
                                          h_stage[:, tt * D:(tt + 1) * D],
                                          dis_sb[:, tt:tt + 1])
                        nc.sync.dma_start(
                            out=agh[:, t0c:t0c + nt_c, :],
                            in_=hsc[:, 0:nt_c, :])
                if step < T_LOOP - 1:
                    if not SIM_MODE:
                        nc.gpsimd.collective_compute(
                            "AllGather", OP.bypass, replica_groups=RG,
                            ins=[ag_h_in.ap().opt()], outs=[h_full.ap().opt()])

            nc.sync.dma_start(out=out_dram[:, :], in_=out_acc[:])

    nc.compile()
    return nc


def kernel(**inputs):
    x = np.asarray(inputs["x"], np.float32)
    edge_index = np.asarray(inputs["edge_index"])
    pp = _preprocess(edge_index)
    nc = _build_kernel(pp)

    nodes = pp["nodes_by_rank"]
    in_maps = []
    for c in range(NCORES):
        vs = nodes[c]
        xsh = np.zeros((SLOTS, F), np.float32)
        real = vs >= 0
        xsh[np.where(real)[0]] = x[vs[real]]
        in_maps.append({
            "xT": np.ascontiguousarray(xsh.T),
            "idxw": np.ascontiguousarray(pp["idxw"][c]),
            "maskNeg": np.ascontiguousarray(pp["maskNeg"][c]),
            "deg_ell": np.ascontiguousarray(pp["deg_ell"][c]),
            "W1": np.asarray(inputs["W1"], np.float32),
            "W2": np.asarray(inputs["W2"], np.float32),
            "Wq": np.asarray(inputs["Wq"], np.float32),
            "Wk": np.asarray(inputs["Wk"], np.float32),
            "b1": np.asarray(inputs["b1"], np.float32),
            "bq": np.asarray(inputs["bq"], np.float32),
            "bk": np.asarray(inputs["bk"], np.float32),
            "b2": np.asarray(inputs["b2"], np.float32),
            "att": np.asarray(inputs["att_logits"], np.float32),
        })

    global LAST_RESULTS
    res = bass_utils.run_bass_kernel_spmd(nc, in_maps,
                                          core_ids=list(range(NCORES)),
                                          trace=TRACE)
    LAST_RESULTS = res

    out = np.zeros((N, D), np.float32)
    for c in range(NCORES):
        oc = np.asarray(res.results[c]["out"]).reshape(P, NT, D)
        vs = nodes[c]
        real = np.where(vs >= 0)[0]
        out[vs[real]] = oc[real % P, real // P]
    return out

